# revision 7
# baseline (speedup 1.0000x reference)
"""MoE (top-2 of 8 experts) Trainium2 kernel, expert-parallel across 8 NeuronCores.

Strategy (matches the expert-parallel sharding hint):
  - Host computes the router (logits -> top-2 -> softmax) and performs the
    token all-to-all: tokens are gathered per expert, padded to a common
    capacity C, and each core gets one expert's tokens + that expert's
    W1/b1/W2 weights.
  - Each core runs a Bass/Tile kernel computing
        y = gelu_exact(x @ W1 + b1) @ W2
    in bf16 (fp32 PSUM accumulate, ~3e-3 rel err, well under the 2e-2 gate).
  - Host scatter-adds the per-expert outputs back with the routing weights
    and adds sum_k w_k * b2[e_k] (folding b2 into the host combine).

Per-core dataflow (two phases, PE never idles between them):
  Phase A (h = gelu(x @ W1 + b1)): stationary = W1 128x128 blocks streamed
  from HBM, moving = xT token chunks; PSUM [f, 512 tok]; exact GELU +
  per-partition bias b1 fused into one ScalarE activation per tile; h kept
  RESIDENT in SBUF as bf16 (32 tiles x [128 f, C tok]).
  Phase B (y = h @ W2): W2 fully resident in SBUF (prefetched during phase
  A); stationary = h blocks [128 f, 128 tok], moving = W2 rows [128 f,
  512 d]; each token tile's y accumulates over all 32 f-tiles in a dedicated
  PSUM bank pair, then drains (ScalarE copy -> bf16 -> DMA) while the next
  token pair accumulates - no SBUF fp32 accumulator, no VectorE adds, and
  the output DMA tail shrinks to one token pair.
"""

import numpy as np
import ml_dtypes

import concourse.bass as bass
import concourse.mybir as mybir
import concourse.tile as tile
from concourse import bacc
from concourse.bass_utils import run_bass_kernel_spmd

P = 128
D = 1024
F = 4096
E = 8
TOP_K = 2
DK = D // P   # 8 contraction tiles for GEMM1
FT = F // P   # 32 f tiles
N_CORES = 8

BF16 = ml_dtypes.bfloat16

_F32 = mybir.dt.float32
_BF16 = mybir.dt.bfloat16

_compiled = {}  # C -> Bacc program


def _token_chunks(C):
    """Split C into 512-token chunks (PSUM-bank-width moving dim)."""
    chunks = []
    off = 0
    while off < C:
        cn = min(512, C - off)
        chunks.append((off, cn))
        off += cn
    return chunks


def _build(C):
    assert C % 256 == 0
    TT = C // P   # token tiles for GEMM2
    chunks = _token_chunks(C)
    NCH = len(chunks)
    nc = bacc.Bacc(None, target_bir_lowering=False)

    # x is chunk-major: [chunk, dk, token-within-chunk] per partition row.
    xt_d = nc.dram_tensor("xt", [P, DK * C], _BF16, kind="ExternalInput")
    w1_d = nc.dram_tensor("w1", [FT, P, DK, P], _BF16, kind="ExternalInput")
    w2_d = nc.dram_tensor("w2", [FT, P, D], _BF16, kind="ExternalInput")
    b1_d = nc.dram_tensor("b1", [P, FT], _F32, kind="ExternalInput")
    y_d = nc.dram_tensor("y", [TT, P, D], _BF16, kind="ExternalOutput")

    W1_LOOK = 4  # W1 tiles in flight ahead of the consuming f-tile

    with tile.TileContext(nc) as tc:
        with (
            tc.tile_pool(name="xpool", bufs=1) as xpool,
            tc.tile_pool(name="cpool", bufs=1) as cpool,
            tc.tile_pool(name="w1pool", bufs=W1_LOOK + 2) as w1pool,
            tc.tile_pool(name="w2pool", bufs=1) as w2pool,
            tc.tile_pool(name="hpool", bufs=1) as hpool,
            tc.tile_pool(name="ypool", bufs=3) as ypool,
            tc.tile_pool(name="hpsum", bufs=2, space="PSUM") as hpsum,
            tc.tile_pool(name="ypsum", bufs=3, space="PSUM") as ypsum,
        ):
            # All input streams ride ONE DMA ring (sync) so the hardware FIFO
            # enforces the demand order. A DMA trigger has no data deps, so
            # the Tile scheduler would front-run it on any idle ring and
            # steal HBM bandwidth from the startup critical path.
            def w1_dma(ft):
                t = w1pool.tile([P, DK, P], _BF16, tag="w1t")
                nc.sync.dma_start(out=t[:], in_=w1_d[ft])
                return t

            # W2 lives in ONE resident SBUF tile (so its buffer release is a
            # single end-of-life semaphore op, not 32 serialized ones on the
            # Tensor queue after the last matmul). Same for h, one tile per
            # 512-token chunk.
            w2_sb = w2pool.tile([P, FT, D], _BF16, name="w2sb")
            h_sb = [
                hpool.tile([P, FT, cn], _BF16, tag=f"hc{ci}", name=f"hc{ci}")
                for ci, (_, cn) in enumerate(chunks)
            ]
            w2_fill = [0]

            def w2_dma():
                k = w2_fill[0]
                if k < FT:
                    nc.sync.dma_start(out=w2_sb[:, k], in_=w2_d[k])
                    w2_fill[0] = k + 1

            # Startup demand order: x chunk 0 (in dk halves, so the first
            # 4-dk accumulation run can start after half the bytes), W1[0],
            # W1[1], b1, the rest of x, then the steady W1 stream + W2
            # prefetch. The x tile is split into dk-halves as separate tiles
            # so the first matmuls only depend on the first DMA.
            xt_lo = []
            xt_hi = []
            pre_w1 = {}
            for ci, (c0, cn) in enumerate(chunks):
                lo = xpool.tile([P, DK // 2, cn], _BF16, tag=f"xl{ci}", name=f"xl{ci}")
                hi = xpool.tile([P, DK // 2, cn], _BF16, tag=f"xh{ci}", name=f"xh{ci}")
                o = c0 * DK
                h0 = o + (DK // 2) * cn
                nc.sync.dma_start(out=lo[:], in_=xt_d[:, o:h0])
                if ci == 0:
                    pre_w1[0] = w1_dma(0)
                nc.sync.dma_start(out=hi[:], in_=xt_d[:, h0 : o + DK * cn])
                xt_lo.append(lo)
                xt_hi.append(hi)
                if ci == 0:
                    pre_w1[1] = w1_dma(1)
                    b1_sb = cpool.tile([P, FT], _F32)
                    nc.sync.dma_start(out=b1_sb[:], in_=b1_d[:])

            # PE warm-up: dummy zero matmuls with no DMA deps run during the
            # initial input-DMA wait, so the HAM clock gate reaches 2.4 GHz
            # before the real stream starts.
            warm = cpool.tile([P, 512], _BF16, tag="warm")
            nc.gpsimd.memset(warm[:], 0.0)
            pw = hpsum.tile([P, 512], _F32, tag="ph")
            for k in range(4):
                nc.tensor.matmul(
                    pw[:], warm[:, :P], warm[:], start=(k == 0), stop=(k == 3)
                )

            for ft in range(2, W1_LOOK):
                pre_w1[ft] = w1_dma(ft)

            def x_mv(ci, dk):
                if dk < DK // 2:
                    return xt_lo[ci][:, dk, :]
                return xt_hi[ci][:, dk - DK // 2, :]

            def gemm1_group(ft, ci, w1t):
                c0, cn = chunks[ci]
                ph = hpsum.tile([P, 512], _F32, tag="ph")
                for dk in range(DK):
                    nc.tensor.matmul(
                        ph[:, :cn],
                        w1t[:, dk, :],
                        x_mv(ci, dk),
                        start=(dk == 0),
                        stop=(dk == DK - 1),
                    )
                nc.scalar.activation(
                    h_sb[ci][:, ft, :],
                    ph[:, :cn],
                    mybir.ActivationFunctionType.Gelu,
                    bias=b1_sb[:, ft : ft + 1],
                    scale=1.0,
                )

            # Phase A. The first two f-tiles run chunk-0 only, so the PE has
            # work while the rest of x is still in flight.
            w1_tiles = {}
            order = [(0, 0), (1, 0)]
            order += [(ft, ci) for ft in range(2) for ci in range(1, NCH)]
            order += [(ft, ci) for ft in range(2, FT) for ci in range(NCH)]
            for ft, ci in order:
                if ft not in w1_tiles:
                    w1_tiles[ft] = pre_w1.pop(ft) if ft in pre_w1 else w1_dma(ft)
                    nxt = ft + W1_LOOK
                    if nxt < FT and nxt not in pre_w1:
                        pre_w1[nxt] = w1_dma(nxt)
                    # W2 prefetch rides the same ring, two tiles per f-tile
                    # so the whole of W2 lands well before phase B.
                    w2_dma()
                    w2_dma()
                gemm1_group(ft, ci, w1_tiles[ft])
                if ci == NCH - 1:
                    del w1_tiles[ft]
            while w2_fill[0] < FT:
                w2_dma()

            # Phase B: token pairs, full 32-step PSUM accumulation per pair.
            for tq in range(TT // 2):
                ci = (tq * 2 * P) // 512  # chunk holding this token pair
                cb = tq * 2 * P - chunks[ci][0]  # base token within chunk
                accs = [
                    ypsum.tile([P, D], _F32, tag="py", name=f"py{tq}_{i}")
                    for i in range(2)
                ]
                for ft in range(FT):
                    for tt2 in range(2):
                        hblk = h_sb[ci][:, ft, cb + tt2 * P : cb + (tt2 + 1) * P]
                        for dh in range(2):
                            nc.tensor.matmul(
                                accs[tt2][:, dh * 512 : (dh + 1) * 512],
                                hblk,
                                w2_sb[:, ft, dh * 512 : (dh + 1) * 512],
                                start=(ft == 0),
                                stop=(ft == FT - 1),
                            )
                for tt2 in range(2):
                    tt = tq * 2 + tt2
                    ysb = ypool.tile([P, D], _BF16, tag="ysb")
                    # Drain the two PSUM banks in parallel on Scalar+Vector.
                    nc.scalar.activation(
                        ysb[:, :512],
                        accs[tt2][:, :512],
                        mybir.ActivationFunctionType.Copy,
                    )
                    nc.vector.tensor_copy(ysb[:, 512:], accs[tt2][:, 512:])
                    eng = nc.sync if tt % 2 == 0 else nc.gpsimd
                    eng.dma_start(out=y_d[tt], in_=ysb[:])

    nc.compile()
    return nc


def _route(xf, Wr, br):
    """Host router: exact top-2 + softmax weights (float64 for stable order)."""
    logits = xf.astype(np.float64) @ Wr.astype(np.float64) + br.astype(np.float64)
    order = np.argsort(-logits, axis=1, kind="stable")
    top2 = order[:, :TOP_K]  # [T, 2]
    v = np.take_along_axis(logits, top2, axis=1)
    v = v - v.max(axis=1, keepdims=True)
    ev = np.exp(v)
    rw = (ev / ev.sum(axis=1, keepdims=True)).astype(np.float32)  # [T, 2]
    return top2, rw


def _run(x, Wr, br, W1, b1, W2, b2, trace=False):
    B, S, d = x.shape
    T = B * S
    xf = np.ascontiguousarray(np.asarray(x, dtype=np.float32).reshape(T, d))

    top2, rw = _route(xf, Wr, br)

    token_lists = []
    weight_lists = []
    max_n = 1
    for e in range(E):
        in_slot0 = top2[:, 0] == e
        in_slot1 = top2[:, 1] == e
        toks = np.nonzero(in_slot0 | in_slot1)[0]
        w = np.where(in_slot0[toks], rw[toks, 0], rw[toks, 1]).astype(np.float32)
        token_lists.append(toks)
        weight_lists.append(w)
        max_n = max(max_n, len(toks))

    # Capacity: balanced mean (rounded up to 128). Pairs beyond it are
    # computed on the host (cheap for near-balanced routing); if the routing
    # is badly imbalanced, raise capacity, but never past C_CAP — the SBUF
    # working set (xT + h + W2) scales with C.
    C_CAP = 1280
    C_max = -(-max_n // P) * P
    C_bal = max(2 * P, -(-(T * TOP_K // E) // P) * P)
    n_spill = sum(max(0, len(t) - C_bal) for t in token_lists)
    C = C_bal if n_spill <= 0.15 * T * TOP_K else min(C_max, max(C_bal, C_CAP))
    C = -(-C // 256) * 256
    spill_lists = [(t[C:], w[C:]) for t, w in zip(token_lists, weight_lists)]
    token_lists = [t[:C] for t in token_lists]
    weight_lists = [w[:C] for w in weight_lists]

    if C not in _compiled:
        _compiled[C] = _build(C)
    nc = _compiled[C]

    # Per-expert weight layouts (see _build DRAM shapes)
    W1 = np.asarray(W1, dtype=np.float32)
    W2 = np.asarray(W2, dtype=np.float32)
    b1 = np.asarray(b1, dtype=np.float32)
    b2 = np.asarray(b2, dtype=np.float32)
    w1h = np.ascontiguousarray(
        W1.reshape(E, DK, P, FT, P).transpose(0, 3, 2, 1, 4)
    ).astype(BF16)  # [E, FT, P(dp), DK, P(fi)]
    w2h = np.ascontiguousarray(W2.reshape(E, FT, P, D)).astype(BF16)  # [E, FT, P(fp), D]
    b1h = np.ascontiguousarray(b1.reshape(E, FT, P).transpose(0, 2, 1))  # [E, P, FT]

    in_maps = []
    for e in range(E):
        toks = token_lists[e]
        xg = np.zeros((C, d), dtype=np.float32)
        xg[: len(toks)] = xf[toks]
        xt = np.empty((P, DK * C), dtype=BF16)
        for c0, cn in _token_chunks(C):
            blk = xg[c0 : c0 + cn].T.reshape(DK, P, cn).transpose(1, 0, 2)
            xt[:, c0 * DK : c0 * DK + DK * cn] = blk.reshape(P, DK * cn).astype(BF16)
        in_maps.append({"xt": xt, "w1": w1h[e], "w2": w2h[e], "b1": b1h[e]})

    res = run_bass_kernel_spmd(
        nc, in_maps, core_ids=list(range(N_CORES)), trace=trace
    )

    # Host combine: out[t] = sum_k rw[t,k] * (y_{e_k}(t) + b2[e_k])
    w_dense = np.zeros((T, E), dtype=np.float32)
    np.put_along_axis(w_dense, top2, rw, axis=1)
    out = w_dense @ b2  # [T, D] bias part
    for e in range(E):
        toks = token_lists[e]
        y = np.asarray(res.results[e]["y"], dtype=np.float32).reshape(C, d)
        out[toks] += weight_lists[e][:, None] * y[: len(toks)]

    # Host-side spill: overflow pairs beyond the device capacity.
    try:
        from scipy.special import erf
    except ImportError:
        import math

        erf = np.vectorize(math.erf, otypes=[np.float32])

    sqrt2 = np.float32(np.sqrt(2.0))
    for e in range(E):
        toks, w = spill_lists[e]
        if len(toks) == 0:
            continue
        hs = xf[toks] @ W1[e] + b1[e]
        hs = 0.5 * hs * (1.0 + erf(hs / sqrt2))
        ys = hs @ W2[e]
        out[toks] += w[:, None] * ys

    return out.reshape(B, S, d).astype(np.float32), res


def kernel(x, Wr, br, W1, b1, W2, b2):
    out, _ = _run(x, Wr, br, W1, b1, W2, b2, trace=False)
    return out
